# revision 58
# baseline (speedup 1.0000x reference)
"""Cross-attention kernel for Trainium2, distributed over 8 NeuronCores.

Sharding: data-parallel over batch (4) x tensor-parallel over head groups (2).
Core c handles batch b = c//2, heads [4g, 4g+4) with g = c%2.

Key structural ideas (vs. a dense implementation):

* Host-side compaction. Masked queries (mask[b,i]=False) all produce the
  SAME output row: softmax over an all-masked row is uniform over all m+1
  positions, so out_i = (sum_j v_j + nv)/(m+1) @ Wo + bo — computed on the
  host. Masked context positions contribute exactly 0 after softmax. The
  device only sees the ~50% active queries and ~50% unmasked context
  columns (null token at column 0), cutting attention work ~4x. Scores,
  exp and the q projection are further narrowed to the true (unpadded)
  query count per i-chunk; attn@v reads of the never-written tail see a
  1.0 pre-fill (= exp(0), what a zero pad query produces), and those
  output rows are discarded on the host.

* bf16 matmul operands everywhere; PSUM accumulation stays fp32. The
  device output is bf16 too (host casts back): tolerance is 2e-2, this
  lands ~6e-3.

* The kernel is paced by the Act engine (exp is Act-only; its busy time
  ~48us is the span floor). Per (i-chunk, head-pair) segment: score
  matmuls + exp run >=2 j-tiles ahead of the attn@v matmuls across
  segment boundaries. All other PE work (k/q/v/out projections) is
  queued as fine-grained filler thunks drained one per score tile so a
  long PE burst never starves the exp stream.

* attn@v uses a FLIPPED layout: out partitions = query i (128 per
  i-subtile), free dim = head dim. That makes the cost-model price per
  av matmul ~65 rows instead of 512, and the softmax denominator (an
  extra 1-wide matmul against the vones column) lands as a per-PARTITION
  scalar: normalization is one reciprocal + one broadcast multiply, not
  a reciprocal/broadcast-matmul/multiply chain. The normalized [i, dh]
  blocks are transposed back to [dh, i] with PE transpose-mode matmuls
  against a DMA'd identity, then copied into Osb for the out projection.

* PSUM budget (8 banks): 3x2 score/projection bufs + 1 bank attn@v
  accumulator + 1 bank shared by the den accumulator and the transpose
  staging tile. Within one bank, only the FIRST matmul of an
  accumulation cycle may set start=True (start clears has_written for
  the whole 2KB zero-region; per-element bits then make each other
  group's first write an overwrite).

* Front: the first k/q projection chunk is split per-dc-half with its
  own tanh so the first scores wait on half the work; bulk inputs
  stream via the Pool engine's SWDGE path in parallel with the HWDGE;
  dummy PE matmuls at t=0 keep the p-state ramp going (full clock after
  3us of continuous execution). Tail: the last i-chunk is the smallest,
  its finalize is pipelined per i-subtile, pending attn@v work is
  drained eagerly, and the final PSUM->SBUF bounce runs on the (by then
  idle) Act engine.
"""

import numpy as np
import ml_dtypes

import concourse.bass as bass
import concourse.tile as tile
from concourse import bacc, bass_utils, mybir

FP = mybir.dt.float32
BF = mybir.dt.bfloat16
AF = mybir.ActivationFunctionType
NPBF = ml_dtypes.bfloat16

B, N, M, DIM = 4, 2048, 2048, 512
HEADS, DH = 8, 64
INNER = HEADS * DH
G = 2          # head groups (tensor-parallel degree)
HG = 4         # heads per group
DG = HG * DH   # 256 dims per group
SCALE = 1.0 / np.sqrt(DH)  # 0.125
VW = DH + 1    # v columns per head incl. ones column (den row)

LAST_RESULTS = None
LAST_NC = None
_CACHE = {}


_SPLIT_SKIP = (
    "InstDrain", "InstUnconditionalBranch", "InstCall",
    "InstEventSemaphore", "InstRegisterMove", "InstDmaTrigger",
)


def _split_multi_waits(nc):
    """TRN2 TPB instruction structs accept only ONE sync wait in walrus
    codegen; extra waits assigned by the Tile scheduler are silently dropped
    from the NEFF, which races on hardware. Hoist all-but-one wait onto
    standalone same-engine InstEventSemaphore instructions (sequencer-only
    waits, the same mechanism the framework itself uses) placed immediately
    before the offending instruction."""
    valid = set(mybir.EngineType) - {mybir.EngineType.Unassigned}
    total = 0
    for bb in nc.m.functions[0].blocks:
        new_insts = []
        for ins in bb.instructions:
            si = ins.sync_info
            if (
                getattr(ins, "engine", None) in valid
                and type(ins).__name__ not in _SPLIT_SKIP
                and si is not None
                and si.on_wait
                and len(si.on_wait) > 1
            ):
                waits = list(si.on_wait)
                for w in waits[:-1]:
                    total += 1
                    ev = mybir.InstEventSemaphore(
                        name=f"evsplit{total}_{ins.name}", ins=[], outs=[])
                    ev.engine = ins.engine
                    ev.sync_info = mybir.SyncInfo(on_wait=[w], on_update=[])
                    nc.inst_map[ev.name] = ev
                    new_insts.append(ev)
                si.on_wait = waits[-1:]
            new_insts.append(ins)
        bb.instructions = new_insts
    return total


def _chunks(total):
    """Split total (a multiple of 128) into <=512-sized 128-multiples,
    greedy descending. A small FINAL chunk is deliberate: the last
    segment's den->outproj->DMA chain is the kernel's serial tail, so
    less work there directly shortens the span."""
    out, off, rem = [], 0, total
    while rem:
        take = min(rem, 512)
        if rem - take == 128:
            take = 384
        out.append((off, take))
        off += take
        rem -= take
    return out


def _build(npi, npj, nact, mact):
    nc = bacc.Bacc("TRN2", debug=False, num_devices=8, enable_partition_id=False)
    d = {}

    def inp(name, shape, dt):
        d[name] = nc.dram_tensor(name, shape, dt, kind="ExternalInput").ap()

    jtc = npj // 128
    inp("xT", [DIM, npi], BF)
    inp("cxT", [DIM, npj], BF)
    inp("wq", [DIM, DG], BF)
    inp("wk", [DIM, DG], BF)
    inp("wv", [DIM, DG], BF)
    inp("wo", [DG, DIM], BF)
    inp("vones", [128, jtc * HG], BF)  # 1 for valid j rows (incl null), 0 pads
    inp("nk", [128, 1], FP)            # tanh(null_key) tiled x2
    inp("nv", [1, DG], BF)             # null_value tiled x4
    inp("ident", [128, 128], BF)       # identity for PE transposes
    d["out"] = nc.dram_tensor("out", [npi, DIM], BF, kind="ExternalOutput").ap()

    with tile.TileContext(nc) as tc:
        _body(tc, d, npi, npj, nact, mact)
    nc.compile()
    return nc


def _body(tc, d, npi, npj, nact, mact):
    nc = tc.nc
    jtc = npj // 128
    ichunks = _chunks(npi)
    jchunks = _chunks(npj)

    with (
        tc.tile_pool(name="consts", bufs=1) as consts,
        tc.tile_pool(name="big", bufs=1) as big,
        tc.tile_pool(name="spool", bufs=8) as spool,
        tc.tile_pool(name="fop", bufs=3) as fop,
        tc.tile_pool(name="dpool", bufs=4) as dpool,
        tc.tile_pool(name="sp", bufs=3, space="PSUM") as sp_ps,
        tc.tile_pool(name="ap", bufs=1, space="PSUM") as ap_ps,
        tc.tile_pool(name="dt", bufs=1, space="PSUM") as dt_ps,
    ):
        # ---- PE warmup: dummy matmuls keep the p-state ramp going while
        # the input DMAs stream (ramp hits full clock after 3us busy).
        # Emitted first so the wsrc memset heads the DVE queue.
        wsrc = consts.tile([128, 256], BF)
        nc.vector.memset(wsrc[:], 0.5)
        wps = sp_ps.tile([128, 2, 512], FP, tag="sp", name="warm")
        for i in range(10):
            nc.tensor.matmul(wps[:, i % 2, 0:256], wsrc[:, 0:128], wsrc[:],
                             start=True, stop=True)

        # pre-fill the score-tile pool where attn@v reads columns the
        # (pad-narrowed) exp never writes: 1.0 = exp(0), what a zero pad
        # query row would produce. Costs nothing: DVE is idle at t=0.
        ices = [min(cs, max(64, nact - off)) for off, cs in ichunks]
        jces = [min(cs, max(64, mact - off)) for off, cs in jchunks]
        m0 = min(ices)
        hi = max((cs for (off, cs), ce in zip(ichunks, ices) if ce < cs),
                 default=m0)
        if m0 < hi:
            for i in range(8):
                t = spool.tile([128, 2, 512], BF, tag="s", name=f"sinit{i}")
                nc.vector.memset(t[:, :, m0:hi], 1.0)

        # ---- inputs. One whole tile per DMA (sliced DMA writes into a
        # shared tile mis-sync at the NEFF level — see module docstring).
        # The chunk-0 k-projection inputs (wk, cx chunk 0) are split
        # per-contraction-chunk into separate tiles so the very first
        # matmul only waits for two small transfers; they are issued
        # first. Later inputs go through the Pool engine's SWDGE path so
        # they don't queue behind the front transfers on the HWDGE.
        cxSrc = d["cxT"].rearrange("(c p) j -> p c j", p=128)
        xSrc = d["xT"].rearrange("(c p) i -> p c i", p=128)
        wk = consts.tile([128, 4, DG], BF)
        nc.sync.dma_start(wk[:], d["wk"].rearrange("(c p) d -> p c d", p=128))
        wq = consts.tile([128, 4, DG], BF)
        nc.sync.dma_start(wq[:], d["wq"].rearrange("(c p) d -> p c d", p=128))
        nk = consts.tile([128, 1], FP)
        nc.sync.dma_start(nk[:], d["nk"])
        # chunk-0 context is DMA'd in two halves so the first k-projection
        # half (and with it the first tanh/exp) only waits for 256 columns
        j0a = min(256, jchunks[0][1])
        j0b = jchunks[0][1] - j0a
        cx0a = big.tile([128, 4, j0a], BF, name="cx0a")
        nc.gpsimd.dma_start(cx0a[:], cxSrc[:, :, 0:j0a])
        cxTt, xTt = [None], []
        t = big.tile([128, 4, ichunks[0][1]], BF, name="xT0")
        nc.gpsimd.dma_start(t[:], xSrc[:, :, 0:ichunks[0][1]])
        xTt.append(t)
        cx0b = None
        if j0b:
            cx0b = big.tile([128, 4, j0b], BF, name="cx0b")
            nc.gpsimd.dma_start(cx0b[:], cxSrc[:, :, j0a:j0a + j0b])
        for c in range(1, max(len(jchunks), len(ichunks))):
            if c < len(jchunks):
                off, cs = jchunks[c]
                t = big.tile([128, 4, cs], BF, name=f"cxT{c}")
                nc.gpsimd.dma_start(t[:], cxSrc[:, :, off:off + cs])
                cxTt.append(t)
            if c < len(ichunks):
                off, cs = ichunks[c]
                t = big.tile([128, 4, cs], BF, name=f"xT{c}")
                nc.gpsimd.dma_start(t[:], xSrc[:, :, off:off + cs])
                xTt.append(t)
        wo = consts.tile([128, 2, DIM], BF)
        nc.gpsimd.dma_start(wo[:], d["wo"].rearrange("(c p) o -> p c o", p=128))

        def cx_loc(j0):
            """Map a global j column offset to (chunk idx, local offset)."""
            for c, (off, cs) in enumerate(jchunks):
                if j0 < off + cs:
                    return c, j0 - off
            raise AssertionError(j0)

        def cx_sl(c, cc, lo, hi):
            """[128, hi-lo] context slice; chunk 0 is split at j0a (callers
            only use 128-aligned slices, which never straddle it)."""
            if c == 0:
                if hi <= j0a:
                    return cx0a[:, cc, lo:hi]
                assert lo >= j0a
                return cx0b[:, cc, lo - j0a:hi - j0a]
            return cxTt[c][:, cc, lo:hi]

        qT = big.tile([128, 2, npi], BF)
        kT = big.tile([128, 2, npj], BF)
        # pad context columns: k projection/tanh are narrowed to the real
        # count, so zero-fill the tail once (score 0 -> exp(0)=1, identical
        # to projecting the zero-padded context; vones keeps these out of
        # the denominator and v rows there are zero)
        mj = jchunks[-1][0] + jces[-1]
        if mj < npj:
            nc.vector.memset(kT[:, :, mj:npj], 0.0)
        vsb = big.tile([128, jtc, HG, VW], BF)
        Osb = big.tile([128, 2, npi], BF)

        # inputs not needed until the attention stream is running; issued
        # after the critical wk/wq/cx0/x0 set so they don't contend for the
        # DMA engines during the prologue
        wv = consts.tile([128, 4, DG], BF)
        nc.sync.dma_start(wv[:], d["wv"].rearrange("(c p) d -> p c d", p=128))
        # vones/nv bounce through whole tiles + engine copies: sliced DMA
        # writes into vsb are not reliably ordered against its readers
        vot = consts.tile([128, jtc * HG], BF)
        nc.sync.dma_start(vot[:], d["vones"])
        nc.vector.tensor_copy(
            vsb[:, :, :, DH:VW],
            vot[:].rearrange("p (j h o) -> p j h o", h=HG, o=1))
        nvt = consts.tile([1, DG], BF)
        nc.sync.dma_start(nvt[:], d["nv"])
        ident = consts.tile([128, 128], BF)
        nc.sync.dma_start(ident[:], d["ident"])

        nic = len(ichunks)
        segs = [(ci, hp) for ci in range(nic) for hp in range(2)]
        po_of = {}


        def kqpart(kind, c, dc):
            """One dc-half of a k/q projection chunk: 4 matmuls + its own
            tanh. Used for chunk 0 so the first scores only wait on half
            the projection work, and as a fine-grained PE filler."""
            ps = sp_ps.tile([128, 2, 512], FP, tag="sp",
                            name=f"ps{kind}{c}d{dc}")
            if kind == "k":
                off, cs = jchunks[c]
                cs = jces[c]
                dst = kT
                ranges = ([(0, min(j0a, cs))] +
                          ([(j0a, cs)] if cs > j0a else [])
                          if c == 0 else [(0, cs)])
                for lo, hi in ranges:
                    for cc in range(4):
                        nc.tensor.matmul(
                            ps[:, dc, lo:hi],
                            wk[:, cc, dc * 128:(dc + 1) * 128],
                            cx_sl(c, cc, lo, hi),
                            start=(cc == 0), stop=(cc == 3),
                        )
            else:
                off, cs = ichunks[c]
                cs = ices[c]
                dst = qT
                for cc in range(4):
                    nc.tensor.matmul(
                        ps[:, dc, :cs],
                        wq[:, cc, dc * 128:(dc + 1) * 128],
                        xTt[c][:, cc, :cs],
                        start=(cc == 0), stop=(cc == 3),
                    )
            nc.scalar.activation(dst[:, dc, off:off + cs], ps[:, dc, :cs],
                                 AF.Tanh)
            if kind == "k" and c == 0:
                # null key column (tanh pre-applied on host) overwrites
                # column 0 after the tanh wrote it
                nc.vector.tensor_copy(kT[:, dc, 0:1], nk[:])

        def kproj(c):
            off, cs = jchunks[c]
            cs = jces[c]
            ps = sp_ps.tile([128, 2, 512], FP, tag="sp", name=f"psk{off}")
            for dc in range(2):
                for cc in range(4):
                    nc.tensor.matmul(
                        ps[:, dc, :cs],
                        wk[:, cc, dc * 128:(dc + 1) * 128],
                        cxTt[c][:, cc, :cs],
                        start=(cc == 0), stop=(cc == 3),
                    )
            nc.scalar.activation(kT[:, :, off:off + cs], ps[:, :, :cs], AF.Tanh)

        def qproj_thunks(ci):
            """qproj split into two PE-filler thunks sharing one psum tile
            and a single tanh (Act instructions are the scarce resource)."""
            off, cs = ichunks[ci]
            cs = ices[ci]
            box = {}

            def half(dc):
                if dc == 0:
                    box["ps"] = sp_ps.tile([128, 2, 512], FP, tag="sp",
                                           name=f"psq{off}")
                ps = box["ps"]
                for cc in range(4):
                    nc.tensor.matmul(
                        ps[:, dc, :cs],
                        wq[:, cc, dc * 128:(dc + 1) * 128],
                        xTt[ci][:, cc, :cs],
                        start=(cc == 0), stop=(cc == 3),
                    )
                if dc == 1:
                    nc.scalar.activation(qT[:, :, off:off + cs],
                                         ps[:, :, :cs], AF.Tanh)
            return [lambda: half(0), lambda: half(1)]

        def vproj_pair(jt0):
            ps = sp_ps.tile([128, 2, 512], FP, tag="sp", name=f"psv{jt0}")
            for s in range(2):
                jt = jt0 + s
                if jt >= jtc:
                    break
                c, loc = cx_loc(jt * 128)
                for cc in range(4):
                    nc.tensor.matmul(
                        ps[:, s, 0:DG],
                        cx_sl(c, cc, loc, loc + 128),
                        wv[:, cc, :],
                        start=(cc == 0), stop=(cc == 3),
                    )
                nc.vector.tensor_copy(
                    vsb[:, jt, :, 0:DH],
                    ps[:, s, 0:DG].rearrange("p (h e) -> p h e", h=HG),
                )
            if jt0 == 0:
                # null token value at j=0 — after the vproj copy of tile 0
                nc.vector.tensor_copy(vsb[0:1, 0, :, 0:DH],
                                      nvt[:].rearrange("a (h e) -> a h e", h=HG))

        def outproj_tile(it, direct=False):
            rows = min(128, nact - it * 128)
            if rows <= 0:
                return
            pf = sp_ps.tile([128, 2, 512], FP, tag="sp", name=f"pf{it}")
            for dc in range(2):
                nc.tensor.matmul(
                    pf[:, 0, :],
                    Osb[:, dc, it * 128:(it + 1) * 128],
                    wo[:, dc, :],
                    start=(dc == 0), stop=(dc == 1),
                )
            fo = fop.tile([128, 512], BF, tag="fo", name=f"fo{it}")
            if direct:
                # kernel tail: Act engine is idle once the exps are done,
                # so the last PSUM->SBUF bounce goes there instead of
                # queueing behind the DVE's finalize work.
                nc.scalar.copy(fo[:], pf[:, 0, :])
            else:
                nc.vector.tensor_copy(fo[:], pf[:, 0, :])
            nc.sync.dma_start(d["out"][it * 128:it * 128 + rows, :],
                              fo[0:rows, :])

        def outproj_thunks(ci, final=False):
            off, cs = ichunks[ci]
            tiles = [off // 128 + t for t in range(cs // 128)]
            return [
                (lambda it=it, d_=final and it == tiles[-1]: outproj_tile(it, d_))
                for it in tiles
            ]

        # ---- attention stream ------------------------------------------
        pend = []   # exp'd score tiles awaiting their attn@v matmuls
        fq = []     # fine-grained PE filler thunks, drained one per S tile

        def flush_fq():
            while fq:
                fq.pop(0)()

        def emit_av(item):
            # attn@v with the FLIPPED layout: out partitions = i (128 per
            # i-subtile), free = dh. Per (i-sub, head): one 64-wide av
            # matmul + one 1-wide den matmul (rhs = the vones column), both
            # accumulating over j tiles. Cheap on PE (cost ~ out free size)
            # and den lands as a per-PARTITION scalar, so normalization is
            # a tensor_scalar instead of a reciprocal+broadcast+mul chain.
            ssb, jt, ci, hp, cbase = item
            off, cs = ichunks[ci]
            K = cs // 128
            if jt == 0:  # lazily created so pool-buffer order == use order
                po_of[(ci, hp)] = (
                    ap_ps.tile([128, 4, 2, DH], FP, tag="ap", name=f"ap{ci}{hp}"),
                    dt_ps.tile([128, 4, 2, 1], FP, tag="dt", name=f"dn{ci}{hp}"),
                )
            apo, dnt = po_of[(ci, hp)]
            last = jt == jtc - 1
            # grouped score tiles put later j-tiles at column base l*ce; an
            # i-subtile read may overhang the 512-wide tile, so trim (the
            # overhung columns are pad queries, discarded on host)
            todo = []
            for isub in range(K):
                for hh in range(2):
                    base = cbase + isub * 128
                    w = min(128, 512 - base)
                    if w > 0:
                        todo.append((isub, hh, base, w))
            for idx, (isub, hh, base, w) in enumerate(todo):
                # start=True clears has_written for the whole 2KB PSUM
                # zero-region (bank), so with several accumulation groups
                # sharing one bank only the FIRST matmul of the whole
                # cycle may set it; per-element has_written bits make
                # every other group's first write an overwrite.
                first = jt == 0 and idx == 0
                lastm = last and idx == len(todo) - 1
                sl = ssb[:, hh, base:base + w]
                nc.tensor.matmul(
                    apo[0:w, isub, hh, :], sl,
                    vsb[:, jt, 2 * hp + hh, 0:DH],
                    start=first, stop=lastm, skip_group_check=True,
                )
                nc.tensor.matmul(
                    dnt[0:w, isub, hh, :], sl,
                    vsb[:, jt, 2 * hp + hh, DH:VW],
                    start=first, stop=lastm, skip_group_check=True,
                )
            if last:
                finalize(ci, hp)

        def finalize(ci, hp):
            # normalize by 1/den (per-partition scalar), transpose each
            # [i, dh] head block back to [dh, i] via the PE, collect into
            # Osb for the output projection.
            off, cs = ichunks[ci]
            K = cs // 128
            apo, dnt = po_of.pop((ci, hp))
            rden = dpool.tile([128, 4, 2, 1], FP, tag="rden")
            nc.vector.reciprocal(rden[:, 0:K, :, :], dnt[:, 0:K, :, :])
            ab = dpool.tile([128, 4, 2, DH], BF, tag="ab")
            trp = dt_ps.tile([128, 4, 128], BF, tag="dt", name=f"tr{ci}{hp}")
            if (ci, hp) == segs[-1]:
                # per-isub pipeline: the first output tile's projection and
                # DMA (the kernel's serial tail) start one isub earlier
                for isub in range(K):
                    nc.vector.tensor_mul(
                        ab[:, isub, :, :], apo[:, isub, :, :],
                        rden[:, isub, :, :].broadcast_to([128, 2, DH]))
                    for hh in range(2):
                        nc.tensor.transpose(
                            trp[64 * hh:64 * (hh + 1), isub, :],
                            ab[:, isub, hh, :], ident[:])
                    nc.vector.tensor_copy(
                        Osb[:, hp, off + isub * 128:off + (isub + 1) * 128],
                        trp[:, isub, :])
            else:
                nc.vector.tensor_mul(
                    ab[:, 0:K, :, :], apo[:, 0:K, :, :],
                    rden[:, 0:K, :, :].broadcast_to([128, K, 2, DH]))
                for isub in range(K):
                    for hh in range(2):
                        nc.tensor.transpose(
                            trp[64 * hh:64 * (hh + 1), isub, :],
                            ab[:, isub, hh, :], ident[:])
                nc.vector.tensor_copy(
                    Osb[:, hp, off:off + cs],
                    trp[:, 0:K, :].rearrange("p a b -> p (a b)"))

        def emit_S_group(ci, hp, jt, g):
            # two j-tiles' scores packed back-to-back in ONE standard score
            # tile (cols [0:ce] and [ce:2ce]) with a single exp: on narrow
            # i-chunks the per-instruction overhead and S->exp->S latency
            # dominate, so batching keeps the Act engine streaming. attn@v
            # still consumes per j-tile via the column base.
            off, cs = ichunks[ci]
            ce = ices[ci]
            sps = sp_ps.tile([128, 2, 512], FP, tag="sp",
                             name=f"sp{ci}_{hp}_{jt}")
            for l in range(g):
                for hh in range(2):
                    nc.tensor.matmul(
                        sps[:, hh, l * ce:(l + 1) * ce],
                        kT[64 * hh:64 * hh + DH, hp,
                           (jt + l) * 128:(jt + l + 1) * 128],
                        qT[64 * hh:64 * hh + DH, hp, off:off + ce],
                        start=True, stop=True,
                    )
            ssb = spool.tile([128, 2, 512], BF, tag="s",
                             name=f"ep{ci}_{hp}_{jt}")
            nc.scalar.activation(ssb[:, :, :g * ce], sps[:, :, :g * ce],
                                 AF.Exp, scale=float(SCALE))
            for l in range(g):
                pend.append((ssb, jt + l, ci, hp, l * ce))
                limit = (1 if (ci, hp) == segs[-1] and jt + l >= jtc - 3
                         else 4)
                while len(pend) > limit:
                    emit_av(pend.pop(0))
            if fq:
                fq.pop(0)()

        def emit_S(ci, hp, jt):
            off, cs = ichunks[ci]
            ce = ices[ci]
            sps = sp_ps.tile([128, 2, 512], FP, tag="sp",
                             name=f"s{ci}_{hp}_{jt}")
            for hh in range(2):
                nc.tensor.matmul(
                    sps[:, hh, :ce],
                    kT[64 * hh:64 * hh + DH, hp, jt * 128:(jt + 1) * 128],
                    qT[64 * hh:64 * hh + DH, hp, off:off + ce],
                    start=True, stop=True,
                )
            ssb = spool.tile([128, 2, 512], BF, tag="s",
                             name=f"e{ci}_{hp}_{jt}")
            nc.scalar.activation(ssb[:, :, :ce], sps[:, :, :ce],
                                 AF.Exp, scale=float(SCALE))
            pend.append((ssb, jt, ci, hp, 0))
            # drain eagerly near the end of the last segment so only the
            # final j-tile's attn@v remains after the last exp
            limit = 1 if (ci, hp) == segs[-1] and jt >= jtc - 3 else 4
            while len(pend) > limit:
                emit_av(pend.pop(0))
            # narrow segments have little Act work per tile; draining a
            # filler between every pair of S matmuls would starve the exp
            # stream, so throttle to every other tile there
            if fq and (ce >= 300 or jt % 2 == 1):
                fq.pop(0)()

        # ---- prologue: half the chunk-0 k/q projections, then stream
        # segment (0,0) with the v projections, the other projection
        # halves, and the remaining kproj chunks interleaved. S matmuls
        # for j-chunk c are only emitted after kproj(c) (PE program order
        # must respect the data dependency or the engines deadlock).
        cs0 = jchunks[0][1]
        psk = sp_ps.tile([128, 2, 512], FP, tag="sp", name="psk0a")
        for cc in range(4):
            nc.tensor.matmul(psk[:, 0, 0:j0a], wk[:, cc, 0:128],
                             cx0a[:, cc, :], start=(cc == 0), stop=(cc == 3))
        nc.scalar.activation(kT[:, 0, 0:j0a], psk[:, 0, 0:j0a], AF.Tanh)
        nc.vector.tensor_copy(kT[:, 0, 0:1], nk[:])
        kqpart("q", 0, 0)
        if j0b:
            psk2 = sp_ps.tile([128, 2, 512], FP, tag="sp", name="psk0b")
            for cc in range(4):
                nc.tensor.matmul(psk2[:, 0, 0:j0b], wk[:, cc, 0:128],
                                 cx0b[:, cc, :], start=(cc == 0),
                                 stop=(cc == 3))
            nc.scalar.activation(kT[:, 0, j0a:cs0], psk2[:, 0, 0:j0b],
                                 AF.Tanh)
        vp_next = 0
        while vp_next < jchunks[0][1] // 128:
            fq.append(lambda p=vp_next: vproj_pair(p))
            vp_next += 2
        fq.append(lambda: kqpart("k", 0, 1))
        fq.append(lambda: kqpart("q", 0, 1))

        s_done = 0
        for _ in range(jchunks[0][1] // 128):
            emit_S(0, 0, s_done)
            s_done += 1
        for c in range(1, len(jchunks)):
            flush_fq()
            kproj(c)
            cover = (jchunks[c][0] + jchunks[c][1]) // 128
            while vp_next < cover:
                fq.append(lambda p=vp_next: vproj_pair(p))
                vp_next += 2
            # only emit S tiles whose kT chunk has been emitted (PE program
            # order must respect the tanh dependency region)
            while s_done < cover:
                emit_S(0, 0, s_done)
                s_done += 1

        # ---- remaining segments. PE fillers (q/out projections) are
        # queued at segment boundaries and drained one thunk per S tile so
        # the Act engine's exp backlog is never exhausted by a long PE
        # burst. outproj(ci) is queued only after finalize(ci, 1)'s
        # emission point (pend drains 4 behind, so finalize(ci, hp) is
        # emitted early in the following segment).
        for ci, hp in segs[1:]:
            flush_fq()
            if hp == 1 and ci + 1 < nic:
                fq.extend(qproj_thunks(ci + 1))
            if hp == 0 and ci >= 2:
                fq.extend(outproj_thunks(ci - 2))
            if hp == 1 and ci == nic - 1 and ci >= 1:
                fq.extend(outproj_thunks(ci - 1))
            n = max(1, 512 // ices[ci])
            if n >= 2:
                jt = 0
                while jt < jtc:
                    g = min(n, jtc - jt)
                    if g > 1:
                        emit_S_group(ci, hp, jt, g)
                    else:
                        emit_S(ci, hp, jt)
                    jt += g
            else:
                for jt in range(jtc):
                    emit_S(ci, hp, jt)
        flush_fq()
        while pend:
            emit_av(pend.pop(0))
        for th in outproj_thunks(nic - 1, final=True):
            th()


def _core_inputs(inputs, core, npi, npj, idx_i, idx_j):
    b, g = core // 2, core % 2
    x = np.asarray(inputs["x"], np.float32)
    context = np.asarray(inputs["context"], np.float32)
    Wq = np.asarray(inputs["Wq"], np.float32)
    Wkv = np.asarray(inputs["Wkv"], np.float32)
    Wo = np.asarray(inputs["Wo"], np.float32)
    null_key = np.asarray(inputs["null_key"], np.float32)
    null_value = np.asarray(inputs["null_value"], np.float32)

    ii, jj = idx_i[b], idx_j[b]
    jtc = npj // 128

    xT = np.zeros((DIM, npi), NPBF)
    xT[:, :len(ii)] = x[b][ii].T
    cxT = np.zeros((DIM, npj), NPBF)
    cxT[:, 1:1 + len(jj)] = context[b][jj].T

    # validity of each j row (incl. null at 0), replicated per head
    valid = (np.arange(npj) < 1 + len(jj)).astype(np.float32)
    vones = np.repeat(valid.reshape(jtc, 128).T[:, :, None], HG, axis=2)

    gs = slice(g * DG, (g + 1) * DG)
    return {
        "xT": xT,
        "cxT": cxT,
        "wq": Wq[:, gs].astype(NPBF),
        "wk": Wkv[:, gs].astype(NPBF),
        "wv": Wkv[:, DIM + g * DG: DIM + (g + 1) * DG].astype(NPBF),
        "wo": Wo[gs, :].astype(NPBF),
        "vones": np.ascontiguousarray(vones.reshape(128, jtc * HG)).astype(NPBF),
        "nk": np.ascontiguousarray(
            np.tanh(np.tile(null_key, 2)).reshape(128, 1)),
        "nv": np.tile(null_value, HG).reshape(1, DG).astype(NPBF),
        "ident": np.eye(128, dtype=np.float32).astype(NPBF),
    }


def kernel(x, context, mask, context_mask, Wq, Wkv, Wo, bo, null_key, null_value):
    global LAST_RESULTS, LAST_NC
    inputs = {
        "x": x, "context": context, "mask": mask, "context_mask": context_mask,
        "Wq": Wq, "Wkv": Wkv, "Wo": Wo, "bo": bo,
        "null_key": null_key, "null_value": null_value,
    }
    mask_np = np.asarray(mask, bool)
    cm_np = np.asarray(context_mask, bool)
    idx_i = [np.nonzero(mask_np[b])[0] for b in range(B)]
    idx_j = [np.nonzero(cm_np[b])[0] for b in range(B)]
    nact = max(len(ii) for ii in idx_i)
    mact = max(1 + len(jj) for jj in idx_j)
    npi = max(128, -(-nact // 128) * 128)
    npj = max(128, -(-mact // 128) * 128)

    key = (npi, npj, nact, mact)
    if key not in _CACHE:
        _CACHE[key] = _build(npi, npj, nact, mact)
    nc = _CACHE[key]
    LAST_NC = nc

    in_maps = [_core_inputs(inputs, core, npi, npj, idx_i, idx_j)
               for core in range(8)]
    res = bass_utils.run_bass_kernel_spmd(nc, in_maps, core_ids=list(range(8)))
    LAST_RESULTS = res

    Wkv_np = np.asarray(Wkv, np.float32)
    Wo_np = np.asarray(Wo, np.float32)
    bo_np = np.asarray(bo, np.float32)
    nv_full = np.tile(np.asarray(null_value, np.float32), HEADS)

    out = np.empty((B, N, DIM), np.float32)
    for b in range(B):
        nact = len(idx_i[b])
        if nact:
            s = (res.results[2 * b]["out"][:nact].astype(np.float32)
                 + res.results[2 * b + 1]["out"][:nact].astype(np.float32)
                 + bo_np)
            out[b][idx_i[b]] = s
        # masked queries attend uniformly over ALL m+1 positions
        vsum = np.asarray(context[b], np.float32).sum(0) @ Wkv_np[:, INNER:]
        urow = (vsum + nv_full) / (M + 1) @ Wo_np + bo_np
        out[b][~mask_np[b]] = urow
    return out



# revision 59
# speedup vs baseline: 1.0027x; 1.0027x over previous
"""Cross-attention kernel for Trainium2, distributed over 8 NeuronCores.

Sharding: data-parallel over batch (4) x tensor-parallel over head groups (2).
Core c handles batch b = c//2, heads [4g, 4g+4) with g = c%2.

Key structural ideas (vs. a dense implementation):

* Host-side compaction. Masked queries (mask[b,i]=False) all produce the
  SAME output row: softmax over an all-masked row is uniform over all m+1
  positions, so out_i = (sum_j v_j + nv)/(m+1) @ Wo + bo — computed on the
  host. Masked context positions contribute exactly 0 after softmax. The
  device only sees the ~50% active queries and ~50% unmasked context
  columns (null token at column 0), cutting attention work ~4x. Scores,
  exp and the q projection are further narrowed to the true (unpadded)
  query count per i-chunk; attn@v reads of the never-written tail see a
  1.0 pre-fill (= exp(0), what a zero pad query produces), and those
  output rows are discarded on the host.

* bf16 matmul operands everywhere; PSUM accumulation stays fp32. The
  device output is bf16 too (host casts back): tolerance is 2e-2, this
  lands ~6e-3.

* The kernel is paced by the Act engine (exp is Act-only; its busy time
  ~48us is the span floor). Per (i-chunk, head-pair) segment: score
  matmuls + exp run >=2 j-tiles ahead of the attn@v matmuls across
  segment boundaries. All other PE work (k/q/v/out projections) is
  queued as fine-grained filler thunks drained one per score tile so a
  long PE burst never starves the exp stream.

* attn@v uses a FLIPPED layout: out partitions = query i (128 per
  i-subtile), free dim = head dim. That makes the cost-model price per
  av matmul ~65 rows instead of 512, and the softmax denominator (an
  extra 1-wide matmul against the vones column) lands as a per-PARTITION
  scalar: normalization is one reciprocal + one broadcast multiply, not
  a reciprocal/broadcast-matmul/multiply chain. The normalized [i, dh]
  blocks are transposed back to [dh, i] with PE transpose-mode matmuls
  against a DMA'd identity, then copied into Osb for the out projection.

* PSUM budget (8 banks): 3x2 score/projection bufs + 1 bank attn@v
  accumulator + 1 bank shared by the den accumulator and the transpose
  staging tile. Within one bank, only the FIRST matmul of an
  accumulation cycle may set start=True (start clears has_written for
  the whole 2KB zero-region; per-element bits then make each other
  group's first write an overwrite).

* Front: the first k/q projection chunk is split per-dc-half with its
  own tanh so the first scores wait on half the work; bulk inputs
  stream via the Pool engine's SWDGE path in parallel with the HWDGE;
  dummy PE matmuls at t=0 keep the p-state ramp going (full clock after
  3us of continuous execution). Tail: the last i-chunk is the smallest,
  its finalize is pipelined per i-subtile, pending attn@v work is
  drained eagerly, and the final PSUM->SBUF bounce runs on the (by then
  idle) Act engine.
"""

import numpy as np
import ml_dtypes

import concourse.bass as bass
import concourse.tile as tile
from concourse import bacc, bass_utils, mybir

FP = mybir.dt.float32
BF = mybir.dt.bfloat16
AF = mybir.ActivationFunctionType
NPBF = ml_dtypes.bfloat16

B, N, M, DIM = 4, 2048, 2048, 512
HEADS, DH = 8, 64
INNER = HEADS * DH
G = 2          # head groups (tensor-parallel degree)
HG = 4         # heads per group
DG = HG * DH   # 256 dims per group
SCALE = 1.0 / np.sqrt(DH)  # 0.125
VW = DH + 1    # v columns per head incl. ones column (den row)

LAST_RESULTS = None
LAST_NC = None
_CACHE = {}


_SPLIT_SKIP = (
    "InstDrain", "InstUnconditionalBranch", "InstCall",
    "InstEventSemaphore", "InstRegisterMove", "InstDmaTrigger",
)


def _split_multi_waits(nc):
    """TRN2 TPB instruction structs accept only ONE sync wait in walrus
    codegen; extra waits assigned by the Tile scheduler are silently dropped
    from the NEFF, which races on hardware. Hoist all-but-one wait onto
    standalone same-engine InstEventSemaphore instructions (sequencer-only
    waits, the same mechanism the framework itself uses) placed immediately
    before the offending instruction."""
    valid = set(mybir.EngineType) - {mybir.EngineType.Unassigned}
    total = 0
    for bb in nc.m.functions[0].blocks:
        new_insts = []
        for ins in bb.instructions:
            si = ins.sync_info
            if (
                getattr(ins, "engine", None) in valid
                and type(ins).__name__ not in _SPLIT_SKIP
                and si is not None
                and si.on_wait
                and len(si.on_wait) > 1
            ):
                waits = list(si.on_wait)
                for w in waits[:-1]:
                    total += 1
                    ev = mybir.InstEventSemaphore(
                        name=f"evsplit{total}_{ins.name}", ins=[], outs=[])
                    ev.engine = ins.engine
                    ev.sync_info = mybir.SyncInfo(on_wait=[w], on_update=[])
                    nc.inst_map[ev.name] = ev
                    new_insts.append(ev)
                si.on_wait = waits[-1:]
            new_insts.append(ins)
        bb.instructions = new_insts
    return total


def _chunks(total):
    """Split total (a multiple of 128) into <=512-sized 128-multiples,
    greedy descending. A small FINAL chunk is deliberate: the last
    segment's den->outproj->DMA chain is the kernel's serial tail, so
    less work there directly shortens the span."""
    out, off, rem = [], 0, total
    while rem:
        take = min(rem, 512)
        if rem - take == 128:
            take = 384
        out.append((off, take))
        off += take
        rem -= take
    return out


def _build(npi, npj, nact, mact):
    nc = bacc.Bacc("TRN2", debug=False, num_devices=8, enable_partition_id=False)
    d = {}

    def inp(name, shape, dt):
        d[name] = nc.dram_tensor(name, shape, dt, kind="ExternalInput").ap()

    jtc = npj // 128
    inp("xT", [DIM, npi], BF)
    inp("cxT", [DIM, npj], BF)
    inp("wq", [DIM, DG], BF)
    inp("wk", [DIM, DG], BF)
    inp("wv", [DIM, DG], BF)
    inp("wo", [DG, DIM], BF)
    inp("vones", [128, jtc * HG], BF)  # 1 for valid j rows (incl null), 0 pads
    inp("nk", [128, 1], FP)            # tanh(null_key) tiled x2
    inp("nv", [1, DG], BF)             # null_value tiled x4
    inp("ident", [128, 128], BF)       # identity for PE transposes
    d["out"] = nc.dram_tensor("out", [npi, DIM], BF, kind="ExternalOutput").ap()

    with tile.TileContext(nc) as tc:
        _body(tc, d, npi, npj, nact, mact)
    nc.compile()
    return nc


def _body(tc, d, npi, npj, nact, mact):
    nc = tc.nc
    jtc = npj // 128
    ichunks = _chunks(npi)
    jchunks = _chunks(npj)

    with (
        tc.tile_pool(name="consts", bufs=1) as consts,
        tc.tile_pool(name="big", bufs=1) as big,
        tc.tile_pool(name="spool", bufs=8) as spool,
        tc.tile_pool(name="fop", bufs=3) as fop,
        tc.tile_pool(name="dpool", bufs=4) as dpool,
        tc.tile_pool(name="sp", bufs=3, space="PSUM") as sp_ps,
        tc.tile_pool(name="ap", bufs=1, space="PSUM") as ap_ps,
        tc.tile_pool(name="dt", bufs=1, space="PSUM") as dt_ps,
    ):
        # ---- PE warmup: dummy matmuls keep the p-state ramp going while
        # the input DMAs stream (ramp hits full clock after 3us busy).
        # Emitted first so the wsrc memset heads the DVE queue.
        wsrc = consts.tile([128, 256], BF)
        nc.vector.memset(wsrc[:], 0.5)
        wps = sp_ps.tile([128, 2, 512], FP, tag="sp", name="warm")
        for i in range(10):
            nc.tensor.matmul(wps[:, i % 2, 0:256], wsrc[:, 0:128], wsrc[:],
                             start=True, stop=True)

        # pre-fill the score-tile pool where attn@v reads columns the
        # (pad-narrowed) exp never writes: 1.0 = exp(0), what a zero pad
        # query row would produce. Costs nothing: DVE is idle at t=0.
        ices = [min(cs, max(64, nact - off)) for off, cs in ichunks]
        jces = [min(cs, max(64, mact - off)) for off, cs in jchunks]
        m0 = min(ices)
        hi = max((cs for (off, cs), ce in zip(ichunks, ices) if ce < cs),
                 default=m0)
        if m0 < hi:
            for i in range(8):
                t = spool.tile([128, 2, 512], BF, tag="s", name=f"sinit{i}")
                nc.vector.memset(t[:, :, m0:hi], 1.0)

        # ---- inputs. One whole tile per DMA (sliced DMA writes into a
        # shared tile mis-sync at the NEFF level — see module docstring).
        # The chunk-0 k-projection inputs (wk, cx chunk 0) are split
        # per-contraction-chunk into separate tiles so the very first
        # matmul only waits for two small transfers; they are issued
        # first. Later inputs go through the Pool engine's SWDGE path so
        # they don't queue behind the front transfers on the HWDGE.
        cxSrc = d["cxT"].rearrange("(c p) j -> p c j", p=128)
        xSrc = d["xT"].rearrange("(c p) i -> p c i", p=128)
        wk = consts.tile([128, 4, DG], BF)
        nc.sync.dma_start(wk[:], d["wk"].rearrange("(c p) d -> p c d", p=128))
        wq = consts.tile([128, 4, DG], BF)
        nc.sync.dma_start(wq[:], d["wq"].rearrange("(c p) d -> p c d", p=128))
        nk = consts.tile([128, 1], FP)
        nc.sync.dma_start(nk[:], d["nk"])
        # chunk-0 context is DMA'd in two halves so the first k-projection
        # half (and with it the first tanh/exp) only waits for 256 columns
        j0a = min(256, jchunks[0][1])
        j0b = jchunks[0][1] - j0a
        cx0a = big.tile([128, 4, j0a], BF, name="cx0a")
        nc.gpsimd.dma_start(cx0a[:], cxSrc[:, :, 0:j0a])
        cxTt, xTt = [None], []
        t = big.tile([128, 4, ichunks[0][1]], BF, name="xT0")
        nc.gpsimd.dma_start(t[:], xSrc[:, :, 0:ichunks[0][1]])
        xTt.append(t)
        cx0b = None
        if j0b:
            cx0b = big.tile([128, 4, j0b], BF, name="cx0b")
            nc.gpsimd.dma_start(cx0b[:], cxSrc[:, :, j0a:j0a + j0b])
        for c in range(1, max(len(jchunks), len(ichunks))):
            if c < len(jchunks):
                off, cs = jchunks[c]
                t = big.tile([128, 4, cs], BF, name=f"cxT{c}")
                nc.gpsimd.dma_start(t[:], cxSrc[:, :, off:off + cs])
                cxTt.append(t)
            if c < len(ichunks):
                off, cs = ichunks[c]
                t = big.tile([128, 4, cs], BF, name=f"xT{c}")
                nc.gpsimd.dma_start(t[:], xSrc[:, :, off:off + cs])
                xTt.append(t)
        wo = consts.tile([128, 2, DIM], BF)
        nc.gpsimd.dma_start(wo[:], d["wo"].rearrange("(c p) o -> p c o", p=128))

        def cx_loc(j0):
            """Map a global j column offset to (chunk idx, local offset)."""
            for c, (off, cs) in enumerate(jchunks):
                if j0 < off + cs:
                    return c, j0 - off
            raise AssertionError(j0)

        def cx_sl(c, cc, lo, hi):
            """[128, hi-lo] context slice; chunk 0 is split at j0a (callers
            only use 128-aligned slices, which never straddle it)."""
            if c == 0:
                if hi <= j0a:
                    return cx0a[:, cc, lo:hi]
                assert lo >= j0a
                return cx0b[:, cc, lo - j0a:hi - j0a]
            return cxTt[c][:, cc, lo:hi]

        qT = big.tile([128, 2, npi], BF)
        kT = big.tile([128, 2, npj], BF)
        # pad context columns: k projection/tanh are narrowed to the real
        # count, so zero-fill the tail once (score 0 -> exp(0)=1, identical
        # to projecting the zero-padded context; vones keeps these out of
        # the denominator and v rows there are zero)
        mj = jchunks[-1][0] + jces[-1]
        if mj < npj:
            nc.vector.memset(kT[:, :, mj:npj], 0.0)
        vsb = big.tile([128, jtc, HG, VW], BF)
        Osb = big.tile([128, 2, npi], BF)

        # inputs not needed until the attention stream is running; issued
        # after the critical wk/wq/cx0/x0 set so they don't contend for the
        # DMA engines during the prologue
        wv = consts.tile([128, 4, DG], BF)
        nc.sync.dma_start(wv[:], d["wv"].rearrange("(c p) d -> p c d", p=128))
        # vones/nv bounce through whole tiles + engine copies: sliced DMA
        # writes into vsb are not reliably ordered against its readers
        vot = consts.tile([128, jtc * HG], BF)
        nc.sync.dma_start(vot[:], d["vones"])
        nc.vector.tensor_copy(
            vsb[:, :, :, DH:VW],
            vot[:].rearrange("p (j h o) -> p j h o", h=HG, o=1))
        nvt = consts.tile([1, DG], BF)
        nc.sync.dma_start(nvt[:], d["nv"])
        ident = consts.tile([128, 128], BF)
        nc.sync.dma_start(ident[:], d["ident"])

        nic = len(ichunks)
        segs = [(ci, hp) for ci in range(nic) for hp in range(2)]
        po_of = {}


        def kqpart(kind, c, dc):
            """One dc-half of a k/q projection chunk: 4 matmuls + its own
            tanh. Used for chunk 0 so the first scores only wait on half
            the projection work, and as a fine-grained PE filler."""
            ps = sp_ps.tile([128, 2, 512], FP, tag="sp",
                            name=f"ps{kind}{c}d{dc}")
            if kind == "k":
                off, cs = jchunks[c]
                cs = jces[c]
                dst = kT
                ranges = ([(0, min(j0a, cs))] +
                          ([(j0a, cs)] if cs > j0a else [])
                          if c == 0 else [(0, cs)])
                for lo, hi in ranges:
                    for cc in range(4):
                        nc.tensor.matmul(
                            ps[:, dc, lo:hi],
                            wk[:, cc, dc * 128:(dc + 1) * 128],
                            cx_sl(c, cc, lo, hi),
                            start=(cc == 0), stop=(cc == 3),
                        )
            else:
                off, cs = ichunks[c]
                cs = ices[c]
                dst = qT
                for cc in range(4):
                    nc.tensor.matmul(
                        ps[:, dc, :cs],
                        wq[:, cc, dc * 128:(dc + 1) * 128],
                        xTt[c][:, cc, :cs],
                        start=(cc == 0), stop=(cc == 3),
                    )
            nc.scalar.activation(dst[:, dc, off:off + cs], ps[:, dc, :cs],
                                 AF.Tanh)
            if kind == "k" and c == 0:
                # null key column (tanh pre-applied on host) overwrites
                # column 0 after the tanh wrote it
                nc.vector.tensor_copy(kT[:, dc, 0:1], nk[:])

        def kproj(c):
            off, cs = jchunks[c]
            cs = jces[c]
            ps = sp_ps.tile([128, 2, 512], FP, tag="sp", name=f"psk{off}")
            for dc in range(2):
                for cc in range(4):
                    nc.tensor.matmul(
                        ps[:, dc, :cs],
                        wk[:, cc, dc * 128:(dc + 1) * 128],
                        cxTt[c][:, cc, :cs],
                        start=(cc == 0), stop=(cc == 3),
                    )
            nc.scalar.activation(kT[:, :, off:off + cs], ps[:, :, :cs], AF.Tanh)

        def qproj_thunks(ci):
            """qproj split into two PE-filler thunks sharing one psum tile
            and a single tanh (Act instructions are the scarce resource)."""
            off, cs = ichunks[ci]
            cs = ices[ci]
            box = {}

            def half(dc):
                if dc == 0:
                    box["ps"] = sp_ps.tile([128, 2, 512], FP, tag="sp",
                                           name=f"psq{off}")
                ps = box["ps"]
                for cc in range(4):
                    nc.tensor.matmul(
                        ps[:, dc, :cs],
                        wq[:, cc, dc * 128:(dc + 1) * 128],
                        xTt[ci][:, cc, :cs],
                        start=(cc == 0), stop=(cc == 3),
                    )
                if dc == 1:
                    nc.scalar.activation(qT[:, :, off:off + cs],
                                         ps[:, :, :cs], AF.Tanh)
            return [lambda: half(0), lambda: half(1)]

        def vproj_pair(jt0):
            ps = sp_ps.tile([128, 2, 512], FP, tag="sp", name=f"psv{jt0}")
            for s in range(2):
                jt = jt0 + s
                if jt >= jtc:
                    break
                c, loc = cx_loc(jt * 128)
                for cc in range(4):
                    nc.tensor.matmul(
                        ps[:, s, 0:DG],
                        cx_sl(c, cc, loc, loc + 128),
                        wv[:, cc, :],
                        start=(cc == 0), stop=(cc == 3),
                    )
                nc.vector.tensor_copy(
                    vsb[:, jt, :, 0:DH],
                    ps[:, s, 0:DG].rearrange("p (h e) -> p h e", h=HG),
                )
            if jt0 == 0:
                # null token value at j=0 — after the vproj copy of tile 0
                nc.vector.tensor_copy(vsb[0:1, 0, :, 0:DH],
                                      nvt[:].rearrange("a (h e) -> a h e", h=HG))

        def outproj_tile(it, direct=False):
            rows = min(128, nact - it * 128)
            if rows <= 0:
                return
            pf = sp_ps.tile([128, 2, 512], FP, tag="sp", name=f"pf{it}")
            for dc in range(2):
                nc.tensor.matmul(
                    pf[:, 0, :],
                    Osb[:, dc, it * 128:(it + 1) * 128],
                    wo[:, dc, :],
                    start=(dc == 0), stop=(dc == 1),
                )
            fo = fop.tile([128, 512], BF, tag="fo", name=f"fo{it}")
            if direct:
                # kernel tail: Act engine is idle once the exps are done,
                # so the last PSUM->SBUF bounce goes there instead of
                # queueing behind the DVE's finalize work.
                nc.scalar.copy(fo[:], pf[:, 0, :])
            else:
                nc.vector.tensor_copy(fo[:], pf[:, 0, :])
            nc.sync.dma_start(d["out"][it * 128:it * 128 + rows, :],
                              fo[0:rows, :])

        def outproj_thunks(ci, final=False):
            off, cs = ichunks[ci]
            tiles = [off // 128 + t for t in range(cs // 128)]
            return [
                (lambda it=it, d_=final and it == tiles[-1]: outproj_tile(it, d_))
                for it in tiles
            ]

        # ---- attention stream ------------------------------------------
        pend = []   # exp'd score tiles awaiting their attn@v matmuls
        fq = []     # fine-grained PE filler thunks, drained one per S tile

        def flush_fq():
            while fq:
                fq.pop(0)()

        def emit_av(item):
            # attn@v with the FLIPPED layout: out partitions = i (128 per
            # i-subtile), free = dh. Per (i-sub, head): one 64-wide av
            # matmul + one 1-wide den matmul (rhs = the vones column), both
            # accumulating over j tiles. Cheap on PE (cost ~ out free size)
            # and den lands as a per-PARTITION scalar, so normalization is
            # a tensor_scalar instead of a reciprocal+broadcast+mul chain.
            ssb, jt, ci, hp, cbase = item
            off, cs = ichunks[ci]
            K = cs // 128
            if jt == 0:  # lazily created so pool-buffer order == use order
                po_of[(ci, hp)] = (
                    ap_ps.tile([128, 4, 2, DH], FP, tag="ap", name=f"ap{ci}{hp}"),
                    dt_ps.tile([128, 4, 2, 1], FP, tag="dt", name=f"dn{ci}{hp}"),
                )
            apo, dnt = po_of[(ci, hp)]
            last = jt == jtc - 1
            # grouped score tiles put later j-tiles at column base l*ce; an
            # i-subtile read may overhang the 512-wide tile, so trim (the
            # overhung columns are pad queries, discarded on host)
            todo = []
            for isub in range(K):
                for hh in range(2):
                    base = cbase + isub * 128
                    w = min(128, 512 - base)
                    if w > 0:
                        todo.append((isub, hh, base, w))
            for idx, (isub, hh, base, w) in enumerate(todo):
                # start=True clears has_written for the whole 2KB PSUM
                # zero-region (bank), so with several accumulation groups
                # sharing one bank only the FIRST matmul of the whole
                # cycle may set it; per-element has_written bits make
                # every other group's first write an overwrite.
                first = jt == 0 and idx == 0
                lastm = last and idx == len(todo) - 1
                sl = ssb[:, hh, base:base + w]
                nc.tensor.matmul(
                    apo[0:w, isub, hh, :], sl,
                    vsb[:, jt, 2 * hp + hh, 0:DH],
                    start=first, stop=lastm, skip_group_check=True,
                )
                nc.tensor.matmul(
                    dnt[0:w, isub, hh, :], sl,
                    vsb[:, jt, 2 * hp + hh, DH:VW],
                    start=first, stop=lastm, skip_group_check=True,
                )
            if last:
                finalize(ci, hp)

        def finalize(ci, hp):
            # normalize by 1/den (per-partition scalar), transpose each
            # [i, dh] head block back to [dh, i] via the PE, collect into
            # Osb for the output projection.
            off, cs = ichunks[ci]
            K = cs // 128
            apo, dnt = po_of.pop((ci, hp))
            rden = dpool.tile([128, 4, 2, 1], FP, tag="rden")
            nc.vector.reciprocal(rden[:, 0:K, :, :], dnt[:, 0:K, :, :])
            ab = dpool.tile([128, 4, 2, DH], BF, tag="ab")
            trp = dt_ps.tile([128, 4, 128], BF, tag="dt", name=f"tr{ci}{hp}")
            if (ci, hp) == segs[-1]:
                # per-isub pipeline: the first output tile's projection and
                # DMA (the kernel's serial tail) start one isub earlier
                for isub in range(K):
                    nc.vector.tensor_mul(
                        ab[:, isub, :, :], apo[:, isub, :, :],
                        rden[:, isub, :, :].broadcast_to([128, 2, DH]))
                    for hh in range(2):
                        nc.tensor.transpose(
                            trp[64 * hh:64 * (hh + 1), isub, :],
                            ab[:, isub, hh, :], ident[:])
                    nc.vector.tensor_copy(
                        Osb[:, hp, off + isub * 128:off + (isub + 1) * 128],
                        trp[:, isub, :])
            else:
                nc.vector.tensor_mul(
                    ab[:, 0:K, :, :], apo[:, 0:K, :, :],
                    rden[:, 0:K, :, :].broadcast_to([128, K, 2, DH]))
                for isub in range(K):
                    for hh in range(2):
                        nc.tensor.transpose(
                            trp[64 * hh:64 * (hh + 1), isub, :],
                            ab[:, isub, hh, :], ident[:])
                nc.vector.tensor_copy(
                    Osb[:, hp, off:off + cs],
                    trp[:, 0:K, :].rearrange("p a b -> p (a b)"))

        def emit_S_group(ci, hp, jt, g):
            # two j-tiles' scores packed back-to-back in ONE standard score
            # tile (cols [0:ce] and [ce:2ce]) with a single exp: on narrow
            # i-chunks the per-instruction overhead and S->exp->S latency
            # dominate, so batching keeps the Act engine streaming. attn@v
            # still consumes per j-tile via the column base.
            off, cs = ichunks[ci]
            ce = ices[ci]
            sps = sp_ps.tile([128, 2, 512], FP, tag="sp",
                             name=f"sp{ci}_{hp}_{jt}")
            for l in range(g):
                for hh in range(2):
                    nc.tensor.matmul(
                        sps[:, hh, l * ce:(l + 1) * ce],
                        kT[64 * hh:64 * hh + DH, hp,
                           (jt + l) * 128:(jt + l + 1) * 128],
                        qT[64 * hh:64 * hh + DH, hp, off:off + ce],
                        start=True, stop=True,
                    )
            ssb = spool.tile([128, 2, 512], BF, tag="s",
                             name=f"ep{ci}_{hp}_{jt}")
            nc.scalar.activation(ssb[:, :, :g * ce], sps[:, :, :g * ce],
                                 AF.Exp, scale=float(SCALE))
            for l in range(g):
                pend.append((ssb, jt + l, ci, hp, l * ce))
                limit = (1 if (ci, hp) == segs[-1] and jt + l >= jtc - 3
                         else 4)
                while len(pend) > limit:
                    emit_av(pend.pop(0))
            for _ in range(2):
                if fq:
                    fq.pop(0)()

        def emit_S(ci, hp, jt):
            off, cs = ichunks[ci]
            ce = ices[ci]
            sps = sp_ps.tile([128, 2, 512], FP, tag="sp",
                             name=f"s{ci}_{hp}_{jt}")
            for hh in range(2):
                nc.tensor.matmul(
                    sps[:, hh, :ce],
                    kT[64 * hh:64 * hh + DH, hp, jt * 128:(jt + 1) * 128],
                    qT[64 * hh:64 * hh + DH, hp, off:off + ce],
                    start=True, stop=True,
                )
            ssb = spool.tile([128, 2, 512], BF, tag="s",
                             name=f"e{ci}_{hp}_{jt}")
            nc.scalar.activation(ssb[:, :, :ce], sps[:, :, :ce],
                                 AF.Exp, scale=float(SCALE))
            pend.append((ssb, jt, ci, hp, 0))
            # drain eagerly near the end of the last segment so only the
            # final j-tile's attn@v remains after the last exp
            limit = 1 if (ci, hp) == segs[-1] and jt >= jtc - 3 else 4
            while len(pend) > limit:
                emit_av(pend.pop(0))
            # narrow segments have little Act work per tile; draining a
            # filler between every pair of S matmuls would starve the exp
            # stream, so throttle to every other tile there
            if fq and (ce >= 300 or jt % 2 == 1):
                fq.pop(0)()

        # ---- prologue: half the chunk-0 k/q projections, then stream
        # segment (0,0) with the v projections, the other projection
        # halves, and the remaining kproj chunks interleaved. S matmuls
        # for j-chunk c are only emitted after kproj(c) (PE program order
        # must respect the data dependency or the engines deadlock).
        cs0 = jchunks[0][1]
        psk = sp_ps.tile([128, 2, 512], FP, tag="sp", name="psk0a")
        for cc in range(4):
            nc.tensor.matmul(psk[:, 0, 0:j0a], wk[:, cc, 0:128],
                             cx0a[:, cc, :], start=(cc == 0), stop=(cc == 3))
        nc.scalar.activation(kT[:, 0, 0:j0a], psk[:, 0, 0:j0a], AF.Tanh)
        nc.vector.tensor_copy(kT[:, 0, 0:1], nk[:])
        kqpart("q", 0, 0)
        if j0b:
            psk2 = sp_ps.tile([128, 2, 512], FP, tag="sp", name="psk0b")
            for cc in range(4):
                nc.tensor.matmul(psk2[:, 0, 0:j0b], wk[:, cc, 0:128],
                                 cx0b[:, cc, :], start=(cc == 0),
                                 stop=(cc == 3))
            nc.scalar.activation(kT[:, 0, j0a:cs0], psk2[:, 0, 0:j0b],
                                 AF.Tanh)
        vp_next = 0
        while vp_next < jchunks[0][1] // 128:
            fq.append(lambda p=vp_next: vproj_pair(p))
            vp_next += 2
        fq.append(lambda: kqpart("k", 0, 1))
        fq.append(lambda: kqpart("q", 0, 1))

        s_done = 0
        for _ in range(jchunks[0][1] // 128):
            emit_S(0, 0, s_done)
            s_done += 1
        for c in range(1, len(jchunks)):
            flush_fq()
            kproj(c)
            cover = (jchunks[c][0] + jchunks[c][1]) // 128
            while vp_next < cover:
                fq.append(lambda p=vp_next: vproj_pair(p))
                vp_next += 2
            # only emit S tiles whose kT chunk has been emitted (PE program
            # order must respect the tanh dependency region)
            while s_done < cover:
                emit_S(0, 0, s_done)
                s_done += 1

        # ---- remaining segments. PE fillers (q/out projections) are
        # queued at segment boundaries and drained one thunk per S tile so
        # the Act engine's exp backlog is never exhausted by a long PE
        # burst. outproj(ci) is queued only after finalize(ci, 1)'s
        # emission point (pend drains 4 behind, so finalize(ci, hp) is
        # emitted early in the following segment).
        for ci, hp in segs[1:]:
            flush_fq()
            if hp == 1 and ci + 1 < nic:
                fq.extend(qproj_thunks(ci + 1))
            if hp == 0 and ci >= 2:
                fq.extend(outproj_thunks(ci - 2))
            if hp == 1 and ci == nic - 1 and ci >= 1:
                fq.extend(outproj_thunks(ci - 1))
            n = max(1, 512 // ices[ci])
            if n >= 2:
                jt = 0
                while jt < jtc:
                    g = min(n, jtc - jt)
                    if g > 1:
                        emit_S_group(ci, hp, jt, g)
                    else:
                        emit_S(ci, hp, jt)
                    jt += g
            else:
                for jt in range(jtc):
                    emit_S(ci, hp, jt)
        flush_fq()
        while pend:
            emit_av(pend.pop(0))
        for th in outproj_thunks(nic - 1, final=True):
            th()


def _core_inputs(inputs, core, npi, npj, idx_i, idx_j):
    b, g = core // 2, core % 2
    x = np.asarray(inputs["x"], np.float32)
    context = np.asarray(inputs["context"], np.float32)
    Wq = np.asarray(inputs["Wq"], np.float32)
    Wkv = np.asarray(inputs["Wkv"], np.float32)
    Wo = np.asarray(inputs["Wo"], np.float32)
    null_key = np.asarray(inputs["null_key"], np.float32)
    null_value = np.asarray(inputs["null_value"], np.float32)

    ii, jj = idx_i[b], idx_j[b]
    jtc = npj // 128

    xT = np.zeros((DIM, npi), NPBF)
    xT[:, :len(ii)] = x[b][ii].T
    cxT = np.zeros((DIM, npj), NPBF)
    cxT[:, 1:1 + len(jj)] = context[b][jj].T

    # validity of each j row (incl. null at 0), replicated per head
    valid = (np.arange(npj) < 1 + len(jj)).astype(np.float32)
    vones = np.repeat(valid.reshape(jtc, 128).T[:, :, None], HG, axis=2)

    gs = slice(g * DG, (g + 1) * DG)
    return {
        "xT": xT,
        "cxT": cxT,
        "wq": Wq[:, gs].astype(NPBF),
        "wk": Wkv[:, gs].astype(NPBF),
        "wv": Wkv[:, DIM + g * DG: DIM + (g + 1) * DG].astype(NPBF),
        "wo": Wo[gs, :].astype(NPBF),
        "vones": np.ascontiguousarray(vones.reshape(128, jtc * HG)).astype(NPBF),
        "nk": np.ascontiguousarray(
            np.tanh(np.tile(null_key, 2)).reshape(128, 1)),
        "nv": np.tile(null_value, HG).reshape(1, DG).astype(NPBF),
        "ident": np.eye(128, dtype=np.float32).astype(NPBF),
    }


def kernel(x, context, mask, context_mask, Wq, Wkv, Wo, bo, null_key, null_value):
    global LAST_RESULTS, LAST_NC
    inputs = {
        "x": x, "context": context, "mask": mask, "context_mask": context_mask,
        "Wq": Wq, "Wkv": Wkv, "Wo": Wo, "bo": bo,
        "null_key": null_key, "null_value": null_value,
    }
    mask_np = np.asarray(mask, bool)
    cm_np = np.asarray(context_mask, bool)
    idx_i = [np.nonzero(mask_np[b])[0] for b in range(B)]
    idx_j = [np.nonzero(cm_np[b])[0] for b in range(B)]
    nact = max(len(ii) for ii in idx_i)
    mact = max(1 + len(jj) for jj in idx_j)
    npi = max(128, -(-nact // 128) * 128)
    npj = max(128, -(-mact // 128) * 128)

    key = (npi, npj, nact, mact)
    if key not in _CACHE:
        _CACHE[key] = _build(npi, npj, nact, mact)
    nc = _CACHE[key]
    LAST_NC = nc

    in_maps = [_core_inputs(inputs, core, npi, npj, idx_i, idx_j)
               for core in range(8)]
    res = bass_utils.run_bass_kernel_spmd(nc, in_maps, core_ids=list(range(8)))
    LAST_RESULTS = res

    Wkv_np = np.asarray(Wkv, np.float32)
    Wo_np = np.asarray(Wo, np.float32)
    bo_np = np.asarray(bo, np.float32)
    nv_full = np.tile(np.asarray(null_value, np.float32), HEADS)

    out = np.empty((B, N, DIM), np.float32)
    for b in range(B):
        nact = len(idx_i[b])
        if nact:
            s = (res.results[2 * b]["out"][:nact].astype(np.float32)
                 + res.results[2 * b + 1]["out"][:nact].astype(np.float32)
                 + bo_np)
            out[b][idx_i[b]] = s
        # masked queries attend uniformly over ALL m+1 positions
        vsum = np.asarray(context[b], np.float32).sum(0) @ Wkv_np[:, INNER:]
        urow = (vsum + nv_full) / (M + 1) @ Wo_np + bo_np
        out[b][~mask_np[b]] = urow
    return out

